# revision 1
# baseline (speedup 1.0000x reference)
"""Trainium2 Bass kernel for gnn_message_passing (nn_Conv_82506321756833).

Computes, for N=50000 nodes / E=800000 edges / H=128:
    xp   = gelu(x @ W1 + b1)
    aggr = segment_sum(xp[src] * bases, dst)
    x    = x_feat + aggr
    y    = gelu(bn1(x @ W2 + b2)); y = gelu(bn2(y @ W3 + b3))
    out  = x + y

Sharding: nodes are partitioned contiguously across 8 cores (graph
parallel); each core owns its node shard and all edges whose dst lands in
the shard.  Edges are bucketed by 128-node destination windows (host-side
sort).  For each window the kernel:

  1. gathers the raw x rows of the edge sources FEATURE-major straight
     from HBM (InstDMAGatherAnt transpose=True, 256B tokens) -- there is
     no precomputed xp table and no phase-A GEMM pass at all;
  2. computes xp = gelu(x_g @ W1) per 128-edge tile, where the matmul
     (lhsT = feature-major gathered tile) transposes to edge-major "for
     free";
  3. multiplies by the (host-presorted, edge-major) bases tile -> msg;
  4. scatter-sums via one-hot matmuls msg.T @ S accumulated in PSUM.  S
     is generated ON-CHIP by a DVE tensor_scalar is_equal of an iota row
     against the per-edge dst-slot id (so no one-hot matrices cross HBM);
  5. runs the 2-layer FFN (BN folded into W2/W3 + bias) on the
     feature-major window block; outputs stream to HBM in fp16, one
     store per 4-window group.

The gather source is x in a partition-blocked [NA, H] layout (node n at
flat row (n%128)*BL + n//128) and indices are the matching host-computed
permutation, split into "lo"/"hi" tile groups for int16 index range (hi
gathers from a +32768-row offset view).
"""

import numpy as np

import concourse.bacc as bacc
import concourse.tile as tile
from concourse import mybir

H = 128
WIN = 128
SPLIT = 32768  # int16 index limit for dma_gather
BN_EPS = 1e-5
F16 = mybir.dt.float16
F32 = mybir.dt.float32
I16 = mybir.dt.int16
GELU = mybir.ActivationFunctionType.Gelu
IS_EQ = mybir.AluOpType.is_equal


def _ceil_to(x, m):
    return (x + m - 1) // m * m


def _wrap16_1w(idx):
    """[L] int16 index list -> [128, L//16] wrapped+replicated (one window)."""
    L = idx.shape[0]
    m = idx.reshape(L // 16, 16).T  # [16, L/16]
    return np.ascontiguousarray(np.tile(m, (8, 1)))  # [128, L/16]


def prep_inputs(x_feat, bases, src, dst, W1, b1, W2, b2, W3, b3,
                g1, be1, m1, v1, g2, be2, m2, v2, ncores=8):
    """Host-side sharding: bucket edges by (dst window, src-range), sort,
    pad each group to a per-window tile count, build per-core input maps."""
    N = x_feat.shape[0]
    assert N % ncores == 0
    NSH = N // ncores
    NW = (NSH + WIN - 1) // WIN
    NPAD = NW * WIN
    NA = _ceil_to(N, 128)
    BL = NA // 128

    x_feat = np.asarray(x_feat, np.float32)
    bases = np.asarray(bases, np.float32)
    src = np.asarray(src, np.int64)
    dst = np.asarray(dst, np.int64)

    # Greedy lo/hi row assignment (quota-normalized): pick the group where
    # the node's cells stay lowest relative to their fair share, to flatten
    # the max-over-cores per-window group counts the shared program pads to.
    cell = (dst // NSH) * NW + (dst % NSH) // WIN
    order0 = np.argsort(src, kind="stable")
    s_sorted = src[order0]
    c_sorted = cell[order0]
    starts = np.searchsorted(s_sorted, np.arange(N))
    ends = np.searchsorted(s_sorted, np.arange(N) + 1)
    tot = np.bincount(cell, minlength=ncores * NW).astype(np.float64)
    frac = SPLIT / NA
    qlo = np.maximum(tot * frac, 1.0)
    qhi = np.maximum(tot * (1.0 - frac), 1.0)
    lo_cnt = np.zeros(ncores * NW)
    hi_cnt = np.zeros(ncores * NW)
    lo_set = np.zeros(N, bool)
    n_lo = n_hi = 0
    cap_lo, cap_hi = SPLIT, NA - SPLIT
    for n in np.argsort(-(ends - starts), kind="stable"):
        uc, mult = np.unique(c_sorted[starts[n]:ends[n]], return_counts=True)
        if len(uc):
            d_lo = np.max((lo_cnt[uc] + mult) / qlo[uc])
            d_hi = np.max((hi_cnt[uc] + mult) / qhi[uc])
        else:
            d_lo, d_hi = 0.0, 1.0
        pick_lo = bool(d_lo <= d_hi)
        if pick_lo and n_lo >= cap_lo:
            pick_lo = False
        if not pick_lo and n_hi >= cap_hi:
            pick_lo = True
        if pick_lo:
            lo_set[n] = True
            n_lo += 1
            if len(uc):
                lo_cnt[uc] += mult
        else:
            n_hi += 1
            if len(uc):
                hi_cnt[uc] += mult
    rowof = np.empty(N, np.int64)
    lo_ids = np.nonzero(lo_set)[0]
    hi_ids = np.nonzero(~lo_set)[0]
    rowof[lo_ids] = np.arange(len(lo_ids))
    rowof[hi_ids] = SPLIT + np.arange(len(hi_ids))

    xb = np.zeros((NA, H), np.float16)
    xb[rowof] = x_feat.astype(np.float16)

    w1h = np.ascontiguousarray(np.asarray(W1, np.float32).astype(np.float16))
    a1 = (np.asarray(g1, np.float32) /
          np.sqrt(np.asarray(v1, np.float32) + BN_EPS))
    a2 = (np.asarray(g2, np.float32) /
          np.sqrt(np.asarray(v2, np.float32) + BN_EPS))
    w2f = np.ascontiguousarray((np.asarray(W2, np.float32) * a1[None, :])
                               .astype(np.float16))
    w3f = np.ascontiguousarray((np.asarray(W3, np.float32) * a2[None, :])
                               .astype(np.float16))
    c2 = ((np.asarray(b2, np.float32) - np.asarray(m1, np.float32)) * a1
          + np.asarray(be1, np.float32)).astype(np.float32).reshape(H, 1)
    c3 = ((np.asarray(b3, np.float32) - np.asarray(m2, np.float32)) * a2
          + np.asarray(be2, np.float32)).astype(np.float32).reshape(H, 1)
    have_b1 = bool(np.any(np.asarray(b1)))
    b1h = np.asarray(b1, np.float32).astype(np.float16).reshape(1, H)

    # Pass 1: per-core edge bucketing + per-window group sizes.
    core_of = dst // NSH
    percore = []
    nlo_all = np.zeros((ncores, NW), np.int64)
    nhi_all = np.zeros((ncores, NW), np.int64)
    for k in range(ncores):
        sel = np.nonzero(core_of == k)[0]
        ld = dst[sel] - k * NSH
        w = ld // WIN
        j = ld % WIN
        # gather index = balanced row assignment
        s = rowof[src[sel]]
        hi = (s >= SPLIT).astype(np.int64)
        key2 = w * 2 + hi
        order = np.lexsort((s, key2))
        w, j, s, hi, key2, sel = (w[order], j[order], s[order], hi[order],
                                  key2[order], sel[order])
        cnt2 = np.bincount(key2, minlength=NW * 2)
        nlo_all[k] = cnt2[0::2]
        nhi_all[k] = cnt2[1::2]
        starts2 = np.zeros(NW * 2, np.int64)
        np.cumsum(cnt2[:-1], out=starts2[1:])
        rank = np.arange(len(w)) - starts2[key2]
        percore.append((w, j, s, hi, rank, sel))

    # Shared (max-over-cores) per-window tile tables: all cores run one
    # program, so the unrolled loop sizes must match across cores.
    TLO = np.maximum((nlo_all.max(axis=0) + 127) // 128, 1)
    THI = (nhi_all.max(axis=0) + 127) // 128
    TW = TLO + THI
    OFF = np.zeros(NW + 1, np.int64)
    np.cumsum(TW, out=OFF[1:])
    LOFF = np.zeros(NW + 1, np.int64)
    np.cumsum(TLO, out=LOFF[1:])
    HOFF = np.zeros(NW + 1, np.int64)
    np.cumsum(THI, out=HOFF[1:])
    GT, LOT, HIT = int(OFF[-1]), int(LOFF[-1]), int(HOFF[-1])

    # Pass 2: build per-core arrays in the shared tile grid.
    in_maps = []
    for k in range(ncores):
        w, j, s, hi, rank, sel = percore[k]
        tin = np.where(hi == 1, TLO[w] + rank // 128, rank // 128)
        gt = OFF[w] + tin
        p = rank % 128

        basf = np.zeros((128, GT * H), np.float16)
        bf3 = basf.reshape(128, GT, H)
        bf3[p, gt, :] = bases[sel].astype(np.float16)
        jd = np.full((128, GT), -1, np.float32)
        jd[p, gt] = j.astype(np.float32)

        ilo = np.zeros((128, LOT * 8), np.int16)
        ihi = np.zeros((128, HIT * 8), np.int16)
        lo_m = hi == 0
        hi_m = hi == 1
        for wi in range(NW):
            buf = np.zeros(int(TLO[wi]) * 128, np.int16)
            m = lo_m & (w == wi)
            buf[rank[m]] = s[m].astype(np.int16)
            ilo[:, int(LOFF[wi]) * 8:int(LOFF[wi + 1]) * 8] = _wrap16_1w(buf)
            if THI[wi]:
                buf = np.zeros(int(THI[wi]) * 128, np.int16)
                m = hi_m & (w == wi)
                buf[rank[m]] = (s[m] - SPLIT).astype(np.int16)
                ihi[:, int(HOFF[wi]) * 8:int(HOFF[wi + 1]) * 8] = \
                    _wrap16_1w(buf)

        xfm = np.zeros((H, NPAD), np.float16)
        xfm[:, :NSH] = x_feat[k * NSH:(k + 1) * NSH].T.astype(np.float16)

        maps = dict(xb=xb, basf=basf, jd=jd, ilod=ilo,
                    w1=w1h, w2=w2f, w3=w3f, c2=c2, c3=c3, xfm=xfm)
        if HIT:
            maps["ihid"] = ihi
        if have_b1:
            maps["b1"] = b1h
        in_maps.append(maps)

    meta = dict(N=N, NSH=NSH, NW=NW, NPAD=NPAD, NA=NA,
                TLO=TLO.tolist(), THI=THI.tolist(),
                OFF=OFF.tolist(), LOFF=LOFF.tolist(), HOFF=HOFF.tolist(),
                GT=GT, LOT=LOT, HIT=HIT, have_b1=have_b1)
    return in_maps, meta


def build_program(meta, ncores=8, act=GELU):
    NA, NW, NPAD = meta["NA"], meta["NW"], meta["NPAD"]
    TLO, THI = meta["TLO"], meta["THI"]
    OFF, LOFF, HOFF = meta["OFF"], meta["LOFF"], meta["HOFF"]
    GT, LOT, HIT = meta["GT"], meta["LOT"], meta["HIT"]
    have_b1 = meta["have_b1"]

    nc = bacc.Bacc("TRN2", target_bir_lowering=False, debug=False,
                   num_devices=ncores)
    xb = nc.dram_tensor("xb", [NA, H], F16, kind="ExternalInput").ap()
    xfm = nc.dram_tensor("xfm", [H, NPAD], F16, kind="ExternalInput").ap()
    basf = nc.dram_tensor("basf", [128, GT * H], F16,
                          kind="ExternalInput").ap()
    jdd = nc.dram_tensor("jd", [128, GT], F32, kind="ExternalInput").ap()
    ilod = nc.dram_tensor("ilod", [128, LOT * 8], I16,
                          kind="ExternalInput").ap()
    ihid = (nc.dram_tensor("ihid", [128, HIT * 8], I16,
                           kind="ExternalInput").ap() if HIT else None)
    w1 = nc.dram_tensor("w1", [H, H], F16, kind="ExternalInput").ap()
    w2 = nc.dram_tensor("w2", [H, H], F16, kind="ExternalInput").ap()
    w3 = nc.dram_tensor("w3", [H, H], F16, kind="ExternalInput").ap()
    c2 = nc.dram_tensor("c2", [H, 1], F32, kind="ExternalInput").ap()
    c3 = nc.dram_tensor("c3", [H, 1], F32, kind="ExternalInput").ap()
    b1 = (nc.dram_tensor("b1", [1, H], F16, kind="ExternalInput").ap()
          if have_b1 else None)
    outd = nc.dram_tensor("out", [H, NPAD], F16, kind="ExternalOutput").ap()

    with tile.TileContext(nc) as tc:
        with (
            tc.tile_pool(name="const", bufs=1) as cpool,
            tc.tile_pool(name="bas", bufs=4) as basp,
            tc.tile_pool(name="gat", bufs=4) as gatp,
            tc.tile_pool(name="xp", bufs=2) as xpp,
            tc.tile_pool(name="msg", bufs=4) as msgp,
            tc.tile_pool(name="st", bufs=4) as stp,
            tc.tile_pool(name="ffn", bufs=2) as ffnp,
            tc.tile_pool(name="og", bufs=2) as ogp,
            tc.tile_pool(name="pxp", bufs=2, space="PSUM") as pxp,
            tc.tile_pool(name="pag", bufs=2, space="PSUM") as pag,
            tc.tile_pool(name="pffn", bufs=2, space="PSUM") as pffn,
        ):
            # constants / resident inputs
            w1t = cpool.tile([H, H], F16, tag="w1")
            nc.sync.dma_start(w1t[:], w1[:])
            w2t = cpool.tile([H, H], F16, tag="w2")
            nc.sync.dma_start(w2t[:], w2[:])
            w3t = cpool.tile([H, H], F16, tag="w3")
            nc.sync.dma_start(w3t[:], w3[:])
            c2t = cpool.tile([H, 1], F32, tag="c2")
            nc.sync.dma_start(c2t[:], c2[:])
            c3t = cpool.tile([H, 1], F32, tag="c3")
            nc.sync.dma_start(c3t[:], c3[:])
            xf_t = cpool.tile([H, NPAD], F16, tag="xfm")
            nc.sync.dma_start(xf_t[:], xfm[:])
            jd_t = cpool.tile([128, GT], F32, tag="jd")
            nc.scalar.dma_start(jd_t[:], jdd[:])
            il_t = cpool.tile([128, LOT * 8], I16, tag="ilo")
            nc.scalar.dma_start(il_t[:], ilod[:])
            if HIT:
                ih_t = cpool.tile([128, HIT * 8], I16, tag="ihi")
                nc.scalar.dma_start(ih_t[:], ihid[:])
            iota_t = cpool.tile([128, 128], I16, tag="iota")
            nc.gpsimd.iota(iota_t[:], [[1, 128]], channel_multiplier=0)
            if have_b1:
                b1t = cpool.tile([1, H], F16, tag="b1")
                nc.sync.dma_start(b1t[:], b1[:])
                onest = cpool.tile([1, H], F16, tag="ones")
                nc.gpsimd.memset(onest[:], 1.0)

            OGW = 8  # windows per output-store group
            out_g = None
            for w in range(NW):
                tlo, thi = TLO[w], THI[w]
                tw = tlo + thi
                go, lo, ho = OFF[w], LOFF[w], HOFF[w]
                r0 = w * 128
                if w % OGW == 0:
                    gw = min(OGW, NW - w)
                    out_g = ogp.tile([H, OGW * 128], F16, tag="og")

                bas_t = basp.tile([128, tw * H], F16, tag="bas")
                beng = nc.sync if w % 2 == 0 else nc.scalar
                beng.dma_start(bas_t[:], basf[:, go * H:(go + tw) * H])

                # gather x rows feature-major: [128 feat, tw*128 edges]
                g_t = gatp.tile([128, tw * H], F16, tag="gat")
                g3 = g_t[:].rearrange("p (o e) -> p o e", o=1)
                nc.gpsimd.dma_gather(g3[:, :, 0:tlo * 128], xb,
                                     il_t[:, lo * 8:(lo + tlo) * 8],
                                     tlo * 128, tlo * 128, H,
                                     transpose=True, single_packet=False)
                if thi:
                    nc.gpsimd.dma_gather(g3[:, :, tlo * 128:tw * 128],
                                         xb[SPLIT:NA, :],
                                         ih_t[:, ho * 8:(ho + thi) * 8],
                                         thi * 128, thi * 128, H,
                                         transpose=True, single_packet=False)

                # xp = gelu(x_g @ W1 [+ b1]) per tile; matmul transposes
                # feature-major lhsT into edge-major PSUM tiles.
                xp_t = xpp.tile([128, tw * H], F16, tag="xp")
                for g0 in range(0, tw, 4):
                    gl = min(4, tw - g0)
                    ps4 = pxp.tile([128, 512], F32, tag="pxp")
                    for t in range(g0, g0 + gl):
                        o0 = (t - g0) * 128
                        nc.tensor.matmul(
                            ps4[:, o0:o0 + 128],
                            g_t[:, t * 128:(t + 1) * 128],
                            w1t[:],
                            start=True, stop=not have_b1)
                        if have_b1:
                            nc.tensor.matmul(
                                ps4[:, o0:o0 + 128],
                                onest[:1, :], b1t[:1, :],
                                start=False, stop=True)
                    nc.scalar.activation(
                        xp_t[:, g0 * 128:(g0 + gl) * 128],
                        ps4[:, :gl * 128], act)

                msg_t = msgp.tile([128, tw * H], F16, tag="msg")
                nc.vector.tensor_mul(msg_t[:], xp_t[:], bas_t[:])

                s_t = stp.tile([128, tw * 128], F16, tag="s")
                for t in range(tw):
                    nc.vector.tensor_scalar(
                        s_t[:, t * 128:(t + 1) * 128], iota_t[:],
                        jd_t[:, go + t:go + t + 1], None, IS_EQ)

                ps_ag = pag.tile([128, 128], F32, tag="pag")
                for t in range(tw):
                    nc.tensor.matmul(
                        ps_ag[:],
                        msg_t[:, t * 128:(t + 1) * 128],
                        s_t[:, t * 128:(t + 1) * 128],
                        start=(t == 0), stop=(t == tw - 1))

                x32_t = ffnp.tile([128, 128], F32, tag="x32")
                nc.vector.tensor_add(x32_t[:], ps_ag[:],
                                     xf_t[:, r0:r0 + 128])
                x16_t = ffnp.tile([128, 128], F16, tag="x16")
                nc.vector.tensor_copy(x16_t[:], x32_t[:])

                ps2 = pffn.tile([128, 128], F32, tag="pffn")
                nc.tensor.matmul(ps2[:], w2t[:], x16_t[:],
                                 start=True, stop=True)
                y1_t = ffnp.tile([128, 128], F16, tag="y1")
                nc.scalar.activation(y1_t[:], ps2[:], act, bias=c2t[:, 0:1])
                ps3 = pffn.tile([128, 128], F32, tag="pffn")
                nc.tensor.matmul(ps3[:], w3t[:], y1_t[:],
                                 start=True, stop=True)
                y2_t = ffnp.tile([128, 128], F32, tag="y2")
                nc.scalar.activation(y2_t[:], ps3[:], act, bias=c3t[:, 0:1])
                oc = (w % OGW) * 128
                nc.vector.tensor_add(out_g[:, oc:oc + 128], y2_t[:],
                                     x32_t[:])
                if w % OGW == OGW - 1 or w == NW - 1:
                    g0 = (w // OGW) * OGW * 128
                    nc.sync.dma_start(outd[:, g0:g0 + gw * 128],
                                      out_g[:, :gw * 128])

    nc.compile()
    return nc


def run_compiled(nc, in_maps, meta, ncores=8, **kw):
    from concourse.bass_utils import run_bass_kernel_spmd
    res = run_bass_kernel_spmd(nc, in_maps, list(range(ncores)), **kw)
    N, NSH = meta["N"], meta["NSH"]
    out = np.empty((N, H), np.float32)
    for k in range(ncores):
        out[k * NSH:(k + 1) * NSH] = \
            res.results[k]["out"][:, :NSH].T.astype(np.float32)
    return out, res


def kernel(**inputs):
    inputs = {k: np.asarray(v) for k, v in inputs.items()}
    in_maps, meta = prep_inputs(**inputs)
    nc = build_program(meta)
    out, _ = run_compiled(nc, in_maps, meta)
    return out



# revision 20
# speedup vs baseline: 151.9225x; 151.9225x over previous
"""Trainium2 Bass kernel for gnn_message_passing (nn_Conv_82506321756833).

Computes, for N=50000 nodes / E=800000 edges / H=128:
    xp   = gelu(x @ W1 + b1)
    aggr = segment_sum(xp[src] * bases, dst)
    x    = x_feat + aggr
    y    = gelu(bn1(x @ W2 + b2)); y = gelu(bn2(y @ W3 + b3))
    out  = x + y

Sharding: nodes are partitioned contiguously across 8 cores (graph
parallel); each core owns its node shard and all edges whose dst lands in
the shard.  Edges are bucketed by 128-node destination windows (host-side
sort).  The kernel runs two phases per core:

Phase A (replicated): stream x feature-major, compute xp = gelu(x@W1+b1)
for ALL nodes, store as a row-table [NA, H] f16 in device DRAM (a DRAM
tile, so the tile framework tracks the RAW dependency into phase B).

Phase B, per 128-node dst window:
  1. dma_gather xp rows EDGE-major (transpose=False -- 256B-token row
     reads; transpose gathers are corrupt on swdge queues != 0) across 4
     SWDGE queues round-robin, 8 window-gathers in flight;
  2. multiply by the (host-presorted, edge-major) bases tile -> msg;
  3. scatter-sum via one-hot matmuls msg.T @ S accumulated in PSUM.  S
     is generated ON-CHIP by one DVE is_equal per window comparing an
     iota row (bcast along tiles) against the per-edge dst-slot id
     (bcast along columns);
  4. runs the 2-layer FFN (BN folded into W2/W3 + bias) on the window
     block; outputs stream to HBM in fp16, one store per 8-window group.

The gather source rows use a host-chosen permutation `rowof` balancing
"lo"/"hi" tile groups for int16 index range (hi gathers read from a
+32768-row offset view).
"""

import numpy as np

import concourse.bacc as bacc
import concourse.tile as tile
from concourse import mybir

H = 128
WIN = 128
SPLIT = 32768  # int16 index limit for dma_gather
BCAST_SGEN = True  # one broadcast tensor_tensor per window vs per-tile
NSWQ = 4  # SWDGE queues for gathers
SCRATCH = 32768  # SWDGE descriptor scratch (bytes/partition)
BN_EPS = 1e-5
F16 = mybir.dt.float16
F32 = mybir.dt.float32
I16 = mybir.dt.int16
GELU = mybir.ActivationFunctionType.Gelu
IS_EQ = mybir.AluOpType.is_equal


def _ceil_to(x, m):
    return (x + m - 1) // m * m


def _wrap16_1w(idx):
    """[L] int16 index list -> [128, L//16] wrapped+replicated (one window)."""
    L = idx.shape[0]
    m = idx.reshape(L // 16, 16).T  # [16, L/16]
    return np.ascontiguousarray(np.tile(m, (8, 1)))  # [128, L/16]


def prep_inputs(x_feat, bases, src, dst, W1, b1, W2, b2, W3, b3,
                g1, be1, m1, v1, g2, be2, m2, v2, ncores=8):
    """Host-side sharding: bucket edges by (dst window, src-range), sort,
    pad each group to a per-window tile count, build per-core input maps."""
    N = x_feat.shape[0]
    assert N % ncores == 0
    NSH = N // ncores
    NW = (NSH + WIN - 1) // WIN
    NPAD = NW * WIN
    NA = _ceil_to(N, 128)
    BL = NA // 128

    x_feat = np.asarray(x_feat, np.float32)
    bases = np.asarray(bases, np.float32)
    src = np.asarray(src, np.int64)
    dst = np.asarray(dst, np.int64)

    # Greedy lo/hi row assignment (quota-normalized): pick the group where
    # the node's cells stay lowest relative to their fair share, to flatten
    # the max-over-cores per-window group counts the shared program pads to.
    cell = (dst // NSH) * NW + (dst % NSH) // WIN
    order0 = np.argsort(src, kind="stable")
    s_sorted = src[order0]
    c_sorted = cell[order0]
    starts = np.searchsorted(s_sorted, np.arange(N))
    ends = np.searchsorted(s_sorted, np.arange(N) + 1)
    tot = np.bincount(cell, minlength=ncores * NW).astype(np.float64)
    frac = SPLIT / NA
    qlo = np.maximum(tot * frac, 1.0)
    qhi = np.maximum(tot * (1.0 - frac), 1.0)
    lo_cnt = np.zeros(ncores * NW)
    hi_cnt = np.zeros(ncores * NW)
    lo_set = np.zeros(N, bool)
    n_lo = n_hi = 0
    cap_lo, cap_hi = SPLIT, NA - SPLIT
    for n in np.argsort(-(ends - starts), kind="stable"):
        uc, mult = np.unique(c_sorted[starts[n]:ends[n]], return_counts=True)
        if len(uc):
            d_lo = np.max((lo_cnt[uc] + mult) / qlo[uc])
            d_hi = np.max((hi_cnt[uc] + mult) / qhi[uc])
        else:
            d_lo, d_hi = 0.0, 1.0
        pick_lo = bool(d_lo <= d_hi)
        if pick_lo and n_lo >= cap_lo:
            pick_lo = False
        if not pick_lo and n_hi >= cap_hi:
            pick_lo = True
        if pick_lo:
            lo_set[n] = True
            n_lo += 1
            if len(uc):
                lo_cnt[uc] += mult
        else:
            n_hi += 1
            if len(uc):
                hi_cnt[uc] += mult
    rowof = np.empty(N, np.int64)
    lo_ids = np.nonzero(lo_set)[0]
    hi_ids = np.nonzero(~lo_set)[0]
    rowof[lo_ids] = np.arange(len(lo_ids))
    rowof[hi_ids] = SPLIT + np.arange(len(hi_ids))

    # x feature-major in gather-row order (phase A input)
    xfa = np.zeros((H, NA), np.float16)
    xfa[:, rowof] = x_feat.T.astype(np.float16)

    w1h = np.ascontiguousarray(np.asarray(W1, np.float32).astype(np.float16))
    a1 = (np.asarray(g1, np.float32) /
          np.sqrt(np.asarray(v1, np.float32) + BN_EPS))
    a2 = (np.asarray(g2, np.float32) /
          np.sqrt(np.asarray(v2, np.float32) + BN_EPS))
    w2f = np.ascontiguousarray((np.asarray(W2, np.float32) * a1[None, :])
                               .astype(np.float16))
    w3f = np.ascontiguousarray((np.asarray(W3, np.float32) * a2[None, :])
                               .astype(np.float16))
    c2 = ((np.asarray(b2, np.float32) - np.asarray(m1, np.float32)) * a1
          + np.asarray(be1, np.float32)).astype(np.float32).reshape(H, 1)
    c3 = ((np.asarray(b3, np.float32) - np.asarray(m2, np.float32)) * a2
          + np.asarray(be2, np.float32)).astype(np.float32).reshape(H, 1)
    have_b1 = bool(np.any(np.asarray(b1)))
    b1h = np.asarray(b1, np.float32).astype(np.float16).reshape(1, H)

    # Pass 1: per-core edge bucketing + per-window group sizes.
    core_of = dst // NSH
    percore = []
    nlo_all = np.zeros((ncores, NW), np.int64)
    nhi_all = np.zeros((ncores, NW), np.int64)
    for k in range(ncores):
        sel = np.nonzero(core_of == k)[0]
        ld = dst[sel] - k * NSH
        w = ld // WIN
        j = ld % WIN
        # gather index = balanced row assignment
        s = rowof[src[sel]]
        hi = (s >= SPLIT).astype(np.int64)
        key2 = w * 2 + hi
        order = np.lexsort((s, key2))
        w, j, s, hi, key2, sel = (w[order], j[order], s[order], hi[order],
                                  key2[order], sel[order])
        cnt2 = np.bincount(key2, minlength=NW * 2)
        nlo_all[k] = cnt2[0::2]
        nhi_all[k] = cnt2[1::2]
        starts2 = np.zeros(NW * 2, np.int64)
        np.cumsum(cnt2[:-1], out=starts2[1:])
        rank = np.arange(len(w)) - starts2[key2]
        percore.append((w, j, s, hi, rank, sel))

    # Shared (max-over-cores) per-window tile tables: all cores run one
    # program, so the unrolled loop sizes must match across cores.
    TLO = np.maximum((nlo_all.max(axis=0) + 127) // 128, 1)
    THI = (nhi_all.max(axis=0) + 127) // 128
    TW = TLO + THI
    OFF = np.zeros(NW + 1, np.int64)
    np.cumsum(TW, out=OFF[1:])
    LOFF = np.zeros(NW + 1, np.int64)
    np.cumsum(TLO, out=LOFF[1:])
    HOFF = np.zeros(NW + 1, np.int64)
    np.cumsum(THI, out=HOFF[1:])
    GT, LOT, HIT = int(OFF[-1]), int(LOFF[-1]), int(HOFF[-1])

    # Pass 2: build per-core arrays in the shared tile grid.
    in_maps = []
    for k in range(ncores):
        w, j, s, hi, rank, sel = percore[k]
        tin = np.where(hi == 1, TLO[w] + rank // 128, rank // 128)
        gt = OFF[w] + tin
        p = rank % 128

        basf = np.zeros((128, GT * H), np.float16)
        bf3 = basf.reshape(128, GT, H)
        bf3[p, gt, :] = bases[sel].astype(np.float16)
        jd = np.full((128, GT), -1, np.float16)
        jd[p, gt] = j.astype(np.float16)

        ilo = np.zeros((128, LOT * 8), np.int16)
        ihi = np.zeros((128, HIT * 8), np.int16)
        lo_m = hi == 0
        hi_m = hi == 1
        for wi in range(NW):
            buf = np.zeros(int(TLO[wi]) * 128, np.int16)
            m = lo_m & (w == wi)
            buf[rank[m]] = s[m].astype(np.int16)
            ilo[:, int(LOFF[wi]) * 8:int(LOFF[wi + 1]) * 8] = _wrap16_1w(buf)
            if THI[wi]:
                buf = np.zeros(int(THI[wi]) * 128, np.int16)
                m = hi_m & (w == wi)
                buf[rank[m]] = (s[m] - SPLIT).astype(np.int16)
                ihi[:, int(HOFF[wi]) * 8:int(HOFF[wi + 1]) * 8] = \
                    _wrap16_1w(buf)

        xfm = np.zeros((H, NPAD), np.float16)
        xfm[:, :NSH] = x_feat[k * NSH:(k + 1) * NSH].T.astype(np.float16)

        maps = dict(xfa=xfa, basf=basf, jd=jd, ilod=ilo,
                    w1=w1h, w2=w2f, w3=w3f, c2=c2, c3=c3, xfm=xfm)
        if HIT:
            maps["ihid"] = ihi
        if have_b1:
            maps["b1"] = b1h
        in_maps.append(maps)

    meta = dict(N=N, NSH=NSH, NW=NW, NPAD=NPAD, NA=NA,
                TLO=TLO.tolist(), THI=THI.tolist(),
                OFF=OFF.tolist(), LOFF=LOFF.tolist(), HOFF=HOFF.tolist(),
                GT=GT, LOT=LOT, HIT=HIT, have_b1=have_b1)
    return in_maps, meta


def build_program(meta, ncores=8, act=GELU):
    NA, NW, NPAD = meta["NA"], meta["NW"], meta["NPAD"]
    TLO, THI = meta["TLO"], meta["THI"]
    OFF, LOFF, HOFF = meta["OFF"], meta["LOFF"], meta["HOFF"]
    GT, LOT, HIT = meta["GT"], meta["LOT"], meta["HIT"]
    have_b1 = meta["have_b1"]
    NB = NA // 128  # phase-A node tiles

    nc = bacc.Bacc("TRN2", target_bir_lowering=False, debug=False,
                   num_devices=ncores, num_swdge_queues=NSWQ,
                   dynamic_dma_scratch_size=SCRATCH)
    xfa = nc.dram_tensor("xfa", [H, NA], F16, kind="ExternalInput").ap()
    xfm = nc.dram_tensor("xfm", [H, NPAD], F16, kind="ExternalInput").ap()
    basf = nc.dram_tensor("basf", [128, GT * H], F16,
                          kind="ExternalInput").ap()
    jdd = nc.dram_tensor("jd", [128, GT], F16, kind="ExternalInput").ap()
    ilod = nc.dram_tensor("ilod", [128, LOT * 8], I16,
                          kind="ExternalInput").ap()
    ihid = (nc.dram_tensor("ihid", [128, HIT * 8], I16,
                           kind="ExternalInput").ap() if HIT else None)
    w1 = nc.dram_tensor("w1", [H, H], F16, kind="ExternalInput").ap()
    w2 = nc.dram_tensor("w2", [H, H], F16, kind="ExternalInput").ap()
    w3 = nc.dram_tensor("w3", [H, H], F16, kind="ExternalInput").ap()
    c2 = nc.dram_tensor("c2", [H, 1], F32, kind="ExternalInput").ap()
    c3 = nc.dram_tensor("c3", [H, 1], F32, kind="ExternalInput").ap()
    b1 = (nc.dram_tensor("b1", [1, H], F16, kind="ExternalInput").ap()
          if have_b1 else None)
    outd = nc.dram_tensor("out", [H, NPAD], F16, kind="ExternalOutput").ap()

    swq = [0]

    def q():
        v = swq[0] % NSWQ
        swq[0] += 1
        return v

    with tile.TileContext(nc) as tc:
        with (
            tc.tile_pool(name="const", bufs=1) as cpool,
            tc.tile_pool(name="xpd", bufs=1, space="DRAM") as xpdp,
            tc.tile_pool(name="xa", bufs=4) as xap,
            tc.tile_pool(name="xps", bufs=4) as xpsp,
            tc.tile_pool(name="bas", bufs=4) as basp,
            tc.tile_pool(name="gat", bufs=8) as gatp,
            tc.tile_pool(name="msg", bufs=4) as msgp,
            tc.tile_pool(name="st", bufs=4) as stp,
            tc.tile_pool(name="ffn", bufs=2) as ffnp,
            tc.tile_pool(name="og", bufs=2) as ogp,
            tc.tile_pool(name="pxp", bufs=4, space="PSUM") as pxp,
            tc.tile_pool(name="pag", bufs=2, space="PSUM") as pag,
            tc.tile_pool(name="pffn", bufs=2, space="PSUM") as pffn,
        ):
            # constants / resident inputs
            w1t = cpool.tile([H, H], F16, tag="w1")
            nc.sync.dma_start(w1t[:], w1[:])
            w2t = cpool.tile([H, H], F16, tag="w2")
            nc.sync.dma_start(w2t[:], w2[:])
            w3t = cpool.tile([H, H], F16, tag="w3")
            nc.sync.dma_start(w3t[:], w3[:])
            c2t = cpool.tile([H, 1], F32, tag="c2")
            nc.sync.dma_start(c2t[:], c2[:])
            c3t = cpool.tile([H, 1], F32, tag="c3")
            nc.sync.dma_start(c3t[:], c3[:])
            xf_t = cpool.tile([H, NPAD], F16, tag="xfm")
            nc.sync.dma_start(xf_t[:], xfm[:])
            jd_t = cpool.tile([128, GT], F16, tag="jd")
            nc.scalar.dma_start(jd_t[:], jdd[:])
            if not BCAST_SGEN:
                jd32_t = cpool.tile([128, GT], F32, tag="jd32")
                nc.vector.tensor_copy(jd32_t[:], jd_t[:])
            il_t = cpool.tile([128, LOT * 8], I16, tag="ilo")
            nc.scalar.dma_start(il_t[:], ilod[:])
            if HIT:
                ih_t = cpool.tile([128, HIT * 8], I16, tag="ihi")
                nc.scalar.dma_start(ih_t[:], ihid[:])
            iota_t = cpool.tile([128, 128], F16, tag="iota")
            nc.gpsimd.iota(iota_t[:], [[1, 128]], channel_multiplier=0,
                           allow_small_or_imprecise_dtypes=True)
            if have_b1:
                b1t = cpool.tile([1, H], F16, tag="b1")
                nc.sync.dma_start(b1t[:], b1[:])
                onest = cpool.tile([1, H], F16, tag="ones")
                nc.gpsimd.memset(onest[:], 1.0)

            # ---- Phase A: xp table = gelu(x @ W1 [+ b1]), all NA rows ----
            xpd = xpdp.tile([NA, H], F16, tag="xpd")
            GRP = 4  # node tiles per PSUM bank
            for g0 in range(0, NB, GRP):
                gl = min(GRP, NB - g0)
                xa_t = xap.tile([128, GRP * 128], F16, tag="xa")
                aeng = nc.sync if (g0 // GRP) % 2 == 0 else nc.scalar
                aeng.dma_start(xa_t[:, :gl * 128],
                               xfa[:, g0 * 128:(g0 + gl) * 128])
                ps = pxp.tile([128, GRP * 128], F32, tag="pxp")
                for b in range(gl):
                    nc.tensor.matmul(
                        ps[:, b * 128:(b + 1) * 128],
                        xa_t[:, b * 128:(b + 1) * 128],
                        w1t[:],
                        start=True, stop=not have_b1)
                    if have_b1:
                        nc.tensor.matmul(
                            ps[:, b * 128:(b + 1) * 128],
                            onest[:1, :], b1t[:1, :],
                            start=False, stop=True)
                xp_t = xpsp.tile([128, GRP * 128], F16, tag="xps")
                nc.scalar.activation(xp_t[:, :gl * 128], ps[:, :gl * 128],
                                     act)
                dst3 = xpd[g0 * 128:(g0 + gl) * 128, :] \
                    .rearrange("(b n) h -> n b h", b=gl)
                src3 = xp_t[:, :gl * 128].rearrange("n (b h) -> n b h", b=gl)
                nc.sync.dma_start(dst3, src3)

            # ---- Phase B: per-window gather/scatter + FFN ----
            xpd_lo = xpd[:]
            xpd_hi = xpd[SPLIT:NA, :]
            OGW = 8  # windows per output-store group
            out_g = None
            for w in range(NW):
                tlo, thi = TLO[w], THI[w]
                tw = tlo + thi
                go, lo, ho = OFF[w], LOFF[w], HOFF[w]
                r0 = w * 128
                if w % OGW == 0:
                    gw = min(OGW, NW - w)
                    out_g = ogp.tile([H, OGW * 128], F16, tag="og")

                bas_t = basp.tile([128, tw * H], F16, tag="bas")
                beng = nc.sync if w % 2 == 0 else nc.scalar
                beng.dma_start(bas_t[:], basf[:, go * H:(go + tw) * H])

                # gather xp rows edge-major: [128 edge, tw tiles, H]
                g_t = gatp.tile([128, tw * H], F16, tag="gat")
                g3 = g_t[:].rearrange("p (t e) -> p t e", t=tw)
                nc.gpsimd.dma_gather(g3[:, 0:tlo, :], xpd_lo,
                                     il_t[:, lo * 8:(lo + tlo) * 8],
                                     tlo * 128, tlo * 128, H,
                                     transpose=False, single_packet=False,
                                     queue_num=q())
                if thi:
                    nc.gpsimd.dma_gather(g3[:, tlo:tw, :], xpd_hi,
                                         ih_t[:, ho * 8:(ho + thi) * 8],
                                         thi * 128, thi * 128, H,
                                         transpose=False, single_packet=False,
                                         queue_num=q())

                msg_t = msgp.tile([128, tw * H], F16, tag="msg")
                nc.vector.tensor_mul(msg_t[:], g_t[:], bas_t[:])

                s_t = stp.tile([128, tw * 128], F16, tag="s")
                if BCAST_SGEN:
                    s3 = s_t[:].rearrange("p (t c) -> p t c", t=tw)
                    io_b = iota_t[:].rearrange("p (o c) -> p o c", o=1) \
                        .broadcast_to([128, tw, 128])
                    jd_b = jd_t[:, go:go + tw] \
                        .rearrange("p (t o) -> p t o", o=1) \
                        .broadcast_to([128, tw, 128])
                    nc.vector.tensor_tensor(s3, io_b, jd_b, IS_EQ)
                else:
                    for t in range(tw):
                        nc.vector.tensor_scalar(
                            s_t[:, t * 128:(t + 1) * 128], iota_t[:],
                            jd32_t[:, go + t:go + t + 1], None, IS_EQ)

                ps_ag = pag.tile([128, 128], F32, tag="pag")
                for t in range(tw):
                    nc.tensor.matmul(
                        ps_ag[:],
                        msg_t[:, t * 128:(t + 1) * 128],
                        s_t[:, t * 128:(t + 1) * 128],
                        start=(t == 0), stop=(t == tw - 1))

                x16_t = ffnp.tile([128, 128], F16, tag="x16")
                nc.vector.tensor_add(x16_t[:], ps_ag[:],
                                     xf_t[:, r0:r0 + 128])

                ps2 = pffn.tile([128, 128], F32, tag="pffn")
                nc.tensor.matmul(ps2[:], w2t[:], x16_t[:],
                                 start=True, stop=True)
                y1_t = ffnp.tile([128, 128], F16, tag="y1")
                nc.scalar.activation(y1_t[:], ps2[:], act, bias=c2t[:, 0:1])
                ps3 = pffn.tile([128, 128], F32, tag="pffn")
                nc.tensor.matmul(ps3[:], w3t[:], y1_t[:],
                                 start=True, stop=True)
                y2_t = ffnp.tile([128, 128], F32, tag="y2")
                nc.scalar.activation(y2_t[:], ps3[:], act, bias=c3t[:, 0:1])
                oc = (w % OGW) * 128
                nc.vector.tensor_add(out_g[:, oc:oc + 128], y2_t[:],
                                     x16_t[:])
                if w % OGW == OGW - 1 or w == NW - 1:
                    g0 = (w // OGW) * OGW * 128
                    nc.sync.dma_start(outd[:, g0:g0 + gw * 128],
                                      out_g[:, :gw * 128])

    nc.compile()
    return nc


def run_compiled(nc, in_maps, meta, ncores=8, **kw):
    from concourse.bass_utils import run_bass_kernel_spmd
    res = run_bass_kernel_spmd(nc, in_maps, list(range(ncores)), **kw)
    N, NSH = meta["N"], meta["NSH"]
    out = np.empty((N, H), np.float32)
    for k in range(ncores):
        out[k * NSH:(k + 1) * NSH] = \
            res.results[k]["out"][:, :NSH].T.astype(np.float32)
    return out, res


def kernel(**inputs):
    inputs = {k: np.asarray(v) for k, v in inputs.items()}
    in_maps, meta = prep_inputs(**inputs)
    nc = build_program(meta)
    out, _ = run_compiled(nc, in_maps, meta)
    return out


# revision 21
# speedup vs baseline: 200.7322x; 1.3213x over previous
"""Trainium2 Bass kernel for gnn_message_passing (nn_Conv_82506321756833).

Computes, for N=50000 nodes / E=800000 edges / H=128:
    xp   = gelu(x @ W1 + b1)
    aggr = segment_sum(xp[src] * bases, dst)
    x    = x_feat + aggr
    y    = gelu(bn1(x @ W2 + b2)); y = gelu(bn2(y @ W3 + b3))
    out  = x + y

Sharding: nodes are partitioned contiguously across 8 cores (graph
parallel); each core owns its node shard and all edges whose dst lands in
the shard.  Edges are bucketed by 128-node destination windows (host-side
sort).  The kernel runs two phases per core:

Phase A (replicated): stream x feature-major, compute xp = gelu(x@W1+b1)
for ALL nodes, store as a row-table [NA, H] f16 in device DRAM (a DRAM
tile, so the tile framework tracks the RAW dependency into phase B).

Phase B, per 128-node dst window:
  1. dma_gather xp rows EDGE-major (transpose=False -- 256B-token row
     reads; transpose gathers are corrupt on swdge queues != 0) across 4
     SWDGE queues round-robin, 8 window-gathers in flight;
  2. multiply by the (host-presorted, edge-major) bases tile -> msg;
  3. scatter-sum via one-hot matmuls msg.T @ S accumulated in PSUM.  S
     is generated ON-CHIP by one DVE is_equal per window comparing an
     iota row (bcast along tiles) against the per-edge dst-slot id
     (bcast along columns);
  4. runs the 2-layer FFN (BN folded into W2/W3 + bias) on the window
     block; outputs stream to HBM in fp16, one store per 8-window group.

The gather source rows use a host-chosen permutation `rowof` balancing
"lo"/"hi" tile groups for int16 index range (hi gathers read from a
+32768-row offset view).
"""

import numpy as np

import concourse.bacc as bacc
import concourse.tile as tile
from concourse import mybir

H = 128
WIN = 128
SPLIT = 32768  # int16 index limit for dma_gather
BCAST_SGEN = True  # one broadcast tensor_tensor per window vs per-tile
NSWQ = 4  # SWDGE queues for gathers
SCRATCH = 32768  # SWDGE descriptor scratch (bytes/partition)
BN_EPS = 1e-5
F16 = mybir.dt.float16
F32 = mybir.dt.float32
I16 = mybir.dt.int16
GELU = mybir.ActivationFunctionType.Gelu
IS_EQ = mybir.AluOpType.is_equal


def _ceil_to(x, m):
    return (x + m - 1) // m * m


def _wrap16_1w(idx):
    """[L] int16 index list -> [128, L//16] wrapped+replicated (one window)."""
    L = idx.shape[0]
    m = idx.reshape(L // 16, 16).T  # [16, L/16]
    return np.ascontiguousarray(np.tile(m, (8, 1)))  # [128, L/16]


def prep_inputs(x_feat, bases, src, dst, W1, b1, W2, b2, W3, b3,
                g1, be1, m1, v1, g2, be2, m2, v2, ncores=8):
    """Host-side sharding: bucket edges by (dst window, src-range), sort,
    pad each group to a per-window tile count, build per-core input maps."""
    N = x_feat.shape[0]
    assert N % ncores == 0
    NSH = N // ncores
    NW = (NSH + WIN - 1) // WIN
    NPAD = NW * WIN
    NA = _ceil_to(N, 128)
    BL = NA // 128

    x_feat = np.asarray(x_feat, np.float32)
    bases = np.asarray(bases, np.float32)
    src = np.asarray(src, np.int64)
    dst = np.asarray(dst, np.int64)

    # Greedy lo/hi row assignment (quota-normalized): pick the group where
    # the node's cells stay lowest relative to their fair share, to flatten
    # the max-over-cores per-window group counts the shared program pads to.
    cell = (dst // NSH) * NW + (dst % NSH) // WIN
    order0 = np.argsort(src, kind="stable")
    s_sorted = src[order0]
    c_sorted = cell[order0]
    starts = np.searchsorted(s_sorted, np.arange(N))
    ends = np.searchsorted(s_sorted, np.arange(N) + 1)
    tot = np.bincount(cell, minlength=ncores * NW).astype(np.float64)
    frac = SPLIT / NA
    qlo = np.maximum(tot * frac, 1.0)
    qhi = np.maximum(tot * (1.0 - frac), 1.0)
    lo_cnt = np.zeros(ncores * NW)
    hi_cnt = np.zeros(ncores * NW)
    lo_set = np.zeros(N, bool)
    n_lo = n_hi = 0
    cap_lo, cap_hi = SPLIT, NA - SPLIT
    for n in np.argsort(-(ends - starts), kind="stable"):
        uc, mult = np.unique(c_sorted[starts[n]:ends[n]], return_counts=True)
        if len(uc):
            d_lo = np.max((lo_cnt[uc] + mult) / qlo[uc])
            d_hi = np.max((hi_cnt[uc] + mult) / qhi[uc])
        else:
            d_lo, d_hi = 0.0, 1.0
        pick_lo = bool(d_lo <= d_hi)
        if pick_lo and n_lo >= cap_lo:
            pick_lo = False
        if not pick_lo and n_hi >= cap_hi:
            pick_lo = True
        if pick_lo:
            lo_set[n] = True
            n_lo += 1
            if len(uc):
                lo_cnt[uc] += mult
        else:
            n_hi += 1
            if len(uc):
                hi_cnt[uc] += mult
    rowof = np.empty(N, np.int64)
    lo_ids = np.nonzero(lo_set)[0]
    hi_ids = np.nonzero(~lo_set)[0]
    rowof[lo_ids] = np.arange(len(lo_ids))
    rowof[hi_ids] = SPLIT + np.arange(len(hi_ids))

    # x feature-major in gather-row order (phase A input)
    xfa = np.zeros((H, NA), np.float16)
    xfa[:, rowof] = x_feat.T.astype(np.float16)

    w1h = np.ascontiguousarray(np.asarray(W1, np.float32).astype(np.float16))
    a1 = (np.asarray(g1, np.float32) /
          np.sqrt(np.asarray(v1, np.float32) + BN_EPS))
    a2 = (np.asarray(g2, np.float32) /
          np.sqrt(np.asarray(v2, np.float32) + BN_EPS))
    w2f = np.ascontiguousarray((np.asarray(W2, np.float32) * a1[None, :])
                               .astype(np.float16))
    w3f = np.ascontiguousarray((np.asarray(W3, np.float32) * a2[None, :])
                               .astype(np.float16))
    c2 = ((np.asarray(b2, np.float32) - np.asarray(m1, np.float32)) * a1
          + np.asarray(be1, np.float32)).astype(np.float32).reshape(H, 1)
    c3 = ((np.asarray(b3, np.float32) - np.asarray(m2, np.float32)) * a2
          + np.asarray(be2, np.float32)).astype(np.float32).reshape(H, 1)
    have_b1 = bool(np.any(np.asarray(b1)))
    b1h = np.asarray(b1, np.float32).astype(np.float16).reshape(1, H)

    # Pass 1: per-core edge bucketing + per-window group sizes.
    core_of = dst // NSH
    percore = []
    nlo_all = np.zeros((ncores, NW), np.int64)
    nhi_all = np.zeros((ncores, NW), np.int64)
    for k in range(ncores):
        sel = np.nonzero(core_of == k)[0]
        ld = dst[sel] - k * NSH
        w = ld // WIN
        j = ld % WIN
        # gather index = balanced row assignment
        s = rowof[src[sel]]
        hi = (s >= SPLIT).astype(np.int64)
        key2 = w * 2 + hi
        order = np.lexsort((s, key2))
        w, j, s, hi, key2, sel = (w[order], j[order], s[order], hi[order],
                                  key2[order], sel[order])
        cnt2 = np.bincount(key2, minlength=NW * 2)
        nlo_all[k] = cnt2[0::2]
        nhi_all[k] = cnt2[1::2]
        starts2 = np.zeros(NW * 2, np.int64)
        np.cumsum(cnt2[:-1], out=starts2[1:])
        rank = np.arange(len(w)) - starts2[key2]
        percore.append((w, j, s, hi, rank, sel))

    # Shared (max-over-cores) per-window tile tables: all cores run one
    # program, so the unrolled loop sizes must match across cores.
    TLO = np.maximum((nlo_all.max(axis=0) + 127) // 128, 1)
    THI = (nhi_all.max(axis=0) + 127) // 128
    TW = TLO + THI
    OFF = np.zeros(NW + 1, np.int64)
    np.cumsum(TW, out=OFF[1:])
    LOFF = np.zeros(NW + 1, np.int64)
    np.cumsum(TLO, out=LOFF[1:])
    HOFF = np.zeros(NW + 1, np.int64)
    np.cumsum(THI, out=HOFF[1:])
    GT, LOT, HIT = int(OFF[-1]), int(LOFF[-1]), int(HOFF[-1])

    # Pass 2: build per-core arrays in the shared tile grid.
    in_maps = []
    for k in range(ncores):
        w, j, s, hi, rank, sel = percore[k]
        tin = np.where(hi == 1, TLO[w] + rank // 128, rank // 128)
        gt = OFF[w] + tin
        p = rank % 128

        basf = np.zeros((128, GT * H), np.float16)
        bf3 = basf.reshape(128, GT, H)
        bf3[p, gt, :] = bases[sel].astype(np.float16)
        jd = np.full((128, GT), -1, np.float16)
        jd[p, gt] = j.astype(np.float16)

        ilo = np.zeros((128, LOT * 8), np.int16)
        ihi = np.zeros((128, HIT * 8), np.int16)
        lo_m = hi == 0
        hi_m = hi == 1
        for wi in range(NW):
            buf = np.zeros(int(TLO[wi]) * 128, np.int16)
            m = lo_m & (w == wi)
            buf[rank[m]] = s[m].astype(np.int16)
            ilo[:, int(LOFF[wi]) * 8:int(LOFF[wi + 1]) * 8] = _wrap16_1w(buf)
            if THI[wi]:
                buf = np.zeros(int(THI[wi]) * 128, np.int16)
                m = hi_m & (w == wi)
                buf[rank[m]] = (s[m] - SPLIT).astype(np.int16)
                ihi[:, int(HOFF[wi]) * 8:int(HOFF[wi + 1]) * 8] = \
                    _wrap16_1w(buf)

        xfm = np.zeros((H, NPAD), np.float16)
        xfm[:, :NSH] = x_feat[k * NSH:(k + 1) * NSH].T.astype(np.float16)

        maps = dict(xfa=xfa, basf=basf, jd=jd, ilod=ilo,
                    w1=w1h, w2=w2f, w3=w3f, c2=c2, c3=c3, xfm=xfm)
        if HIT:
            maps["ihid"] = ihi
        if have_b1:
            maps["b1"] = b1h
        in_maps.append(maps)

    meta = dict(N=N, NSH=NSH, NW=NW, NPAD=NPAD, NA=NA,
                TLO=TLO.tolist(), THI=THI.tolist(),
                OFF=OFF.tolist(), LOFF=LOFF.tolist(), HOFF=HOFF.tolist(),
                GT=GT, LOT=LOT, HIT=HIT, have_b1=have_b1)
    return in_maps, meta


def build_program(meta, ncores=8, act=GELU):
    NA, NW, NPAD = meta["NA"], meta["NW"], meta["NPAD"]
    TLO, THI = meta["TLO"], meta["THI"]
    OFF, LOFF, HOFF = meta["OFF"], meta["LOFF"], meta["HOFF"]
    GT, LOT, HIT = meta["GT"], meta["LOT"], meta["HIT"]
    have_b1 = meta["have_b1"]
    NB = NA // 128  # phase-A node tiles

    nc = bacc.Bacc("TRN2", target_bir_lowering=False, debug=False,
                   num_devices=ncores, num_swdge_queues=NSWQ,
                   dynamic_dma_scratch_size=SCRATCH)
    xfa = nc.dram_tensor("xfa", [H, NA], F16, kind="ExternalInput").ap()
    xfm = nc.dram_tensor("xfm", [H, NPAD], F16, kind="ExternalInput").ap()
    basf = nc.dram_tensor("basf", [128, GT * H], F16,
                          kind="ExternalInput").ap()
    jdd = nc.dram_tensor("jd", [128, GT], F16, kind="ExternalInput").ap()
    ilod = nc.dram_tensor("ilod", [128, LOT * 8], I16,
                          kind="ExternalInput").ap()
    ihid = (nc.dram_tensor("ihid", [128, HIT * 8], I16,
                           kind="ExternalInput").ap() if HIT else None)
    w1 = nc.dram_tensor("w1", [H, H], F16, kind="ExternalInput").ap()
    w2 = nc.dram_tensor("w2", [H, H], F16, kind="ExternalInput").ap()
    w3 = nc.dram_tensor("w3", [H, H], F16, kind="ExternalInput").ap()
    c2 = nc.dram_tensor("c2", [H, 1], F32, kind="ExternalInput").ap()
    c3 = nc.dram_tensor("c3", [H, 1], F32, kind="ExternalInput").ap()
    b1 = (nc.dram_tensor("b1", [1, H], F16, kind="ExternalInput").ap()
          if have_b1 else None)
    outd = nc.dram_tensor("out", [H, NPAD], F16, kind="ExternalOutput").ap()

    swq = [0]

    def q():
        v = swq[0] % NSWQ
        swq[0] += 1
        return v

    with tile.TileContext(nc) as tc:
        with (
            tc.tile_pool(name="const", bufs=1) as cpool,
            tc.tile_pool(name="xpd", bufs=1, space="DRAM") as xpdp,
            tc.tile_pool(name="xa", bufs=4) as xap,
            tc.tile_pool(name="xps", bufs=4) as xpsp,
            tc.tile_pool(name="bas", bufs=4) as basp,
            tc.tile_pool(name="gat", bufs=8) as gatp,
            tc.tile_pool(name="msg", bufs=4) as msgp,
            tc.tile_pool(name="st", bufs=4) as stp,
            tc.tile_pool(name="ffn", bufs=2) as ffnp,
            tc.tile_pool(name="og", bufs=2) as ogp,
            tc.tile_pool(name="pxp", bufs=4, space="PSUM") as pxp,
            tc.tile_pool(name="pag", bufs=2, space="PSUM") as pag,
            tc.tile_pool(name="pffn", bufs=2, space="PSUM") as pffn,
        ):
            # constants / resident inputs
            w1t = cpool.tile([H, H], F16, tag="w1")
            nc.sync.dma_start(w1t[:], w1[:])
            w2t = cpool.tile([H, H], F16, tag="w2")
            nc.sync.dma_start(w2t[:], w2[:])
            w3t = cpool.tile([H, H], F16, tag="w3")
            nc.sync.dma_start(w3t[:], w3[:])
            c2t = cpool.tile([H, 1], F32, tag="c2")
            nc.sync.dma_start(c2t[:], c2[:])
            c3t = cpool.tile([H, 1], F32, tag="c3")
            nc.sync.dma_start(c3t[:], c3[:])
            xf_t = cpool.tile([H, NPAD], F16, tag="xfm")
            nc.sync.dma_start(xf_t[:], xfm[:])
            jd_t = cpool.tile([128, GT], F16, tag="jd")
            nc.scalar.dma_start(jd_t[:], jdd[:])
            if not BCAST_SGEN:
                jd32_t = cpool.tile([128, GT], F32, tag="jd32")
                nc.vector.tensor_copy(jd32_t[:], jd_t[:])
            il_t = cpool.tile([128, LOT * 8], I16, tag="ilo")
            nc.scalar.dma_start(il_t[:], ilod[:])
            if HIT:
                ih_t = cpool.tile([128, HIT * 8], I16, tag="ihi")
                nc.scalar.dma_start(ih_t[:], ihid[:])
            iota_t = cpool.tile([128, 128], F16, tag="iota")
            nc.gpsimd.iota(iota_t[:], [[1, 128]], channel_multiplier=0,
                           allow_small_or_imprecise_dtypes=True)
            if have_b1:
                b1t = cpool.tile([1, H], F16, tag="b1")
                nc.sync.dma_start(b1t[:], b1[:])
                onest = cpool.tile([1, H], F16, tag="ones")
                nc.gpsimd.memset(onest[:], 1.0)

            # ---- Phase A: xp table = gelu(x @ W1 [+ b1]), all NA rows ----
            xpd = xpdp.tile([NA, H], F16, tag="xpd")
            GRP = 4  # node tiles per PSUM bank
            for g0 in range(0, NB, GRP):
                gl = min(GRP, NB - g0)
                xa_t = xap.tile([128, GRP * 128], F16, tag="xa")
                aeng = nc.sync if (g0 // GRP) % 2 == 0 else nc.scalar
                aeng.dma_start(xa_t[:, :gl * 128],
                               xfa[:, g0 * 128:(g0 + gl) * 128])
                ps = pxp.tile([128, GRP * 128], F32, tag="pxp")
                for b in range(gl):
                    nc.tensor.matmul(
                        ps[:, b * 128:(b + 1) * 128],
                        xa_t[:, b * 128:(b + 1) * 128],
                        w1t[:],
                        start=True, stop=not have_b1)
                    if have_b1:
                        nc.tensor.matmul(
                            ps[:, b * 128:(b + 1) * 128],
                            onest[:1, :], b1t[:1, :],
                            start=False, stop=True)
                xp_t = xpsp.tile([128, GRP * 128], F16, tag="xps")
                nc.scalar.activation(xp_t[:, :gl * 128], ps[:, :gl * 128],
                                     act)
                dst3 = xpd[g0 * 128:(g0 + gl) * 128, :] \
                    .rearrange("(b n) h -> n b h", b=gl)
                src3 = xp_t[:, :gl * 128].rearrange("n (b h) -> n b h", b=gl)
                nc.sync.dma_start(dst3, src3)

            # ---- Phase B: two passes (lo rows, then hi rows) so lo
            # gathers start as soon as the lo half of xpd is written.
            # Window aggregates live in an SBUF accumulator (f32), with
            # the residual x pre-added during the lo pass.
            xpd_lo = xpd[0:SPLIT, :]
            xpd_hi = xpd[SPLIT:NA, :]
            aggr_t = cpool.tile([128, NPAD], F32, tag="aggr")

            def scatter_pass(w, nt, toff, xsrc, idx_t, ioff):
                """Gather nt tiles of window w (tile offset toff within the
                window group), multiply by bases, one-hot scatter into a
                PSUM tile; returns the PSUM tile."""
                go = OFF[w] + toff
                bas_t = basp.tile([128, nt * H], F16, tag="bas")
                beng = nc.sync if w % 2 == 0 else nc.scalar
                beng.dma_start(bas_t[:], basf[:, go * H:(go + nt) * H])

                g_t = gatp.tile([128, nt * H], F16, tag="gat")
                g3 = g_t[:].rearrange("p (t e) -> p t e", t=nt)
                nc.gpsimd.dma_gather(g3[:, :, :], xsrc,
                                     idx_t[:, ioff * 8:(ioff + nt) * 8],
                                     nt * 128, nt * 128, H,
                                     transpose=False, single_packet=False,
                                     queue_num=q())

                msg_t = msgp.tile([128, nt * H], F16, tag="msg")
                nc.vector.tensor_mul(msg_t[:], g_t[:], bas_t[:])

                s_t = stp.tile([128, nt * 128], F16, tag="s")
                if BCAST_SGEN:
                    s3 = s_t[:].rearrange("p (t c) -> p t c", t=nt)
                    io_b = iota_t[:].rearrange("p (o c) -> p o c", o=1) \
                        .broadcast_to([128, nt, 128])
                    jd_b = jd_t[:, go:go + nt] \
                        .rearrange("p (t o) -> p t o", o=1) \
                        .broadcast_to([128, nt, 128])
                    nc.vector.tensor_tensor(s3, io_b, jd_b, IS_EQ)
                else:
                    for t in range(nt):
                        nc.vector.tensor_scalar(
                            s_t[:, t * 128:(t + 1) * 128], iota_t[:],
                            jd32_t[:, go + t:go + t + 1], None, IS_EQ)

                ps_ag = pag.tile([128, 128], F32, tag="pag")
                for t in range(nt):
                    nc.tensor.matmul(
                        ps_ag[:],
                        msg_t[:, t * 128:(t + 1) * 128],
                        s_t[:, t * 128:(t + 1) * 128],
                        start=(t == 0), stop=(t == nt - 1))
                return ps_ag

            # Pass 1: lo rows. aggr <- psum_lo + x_residual
            for w in range(NW):
                ps = scatter_pass(w, TLO[w], 0, xpd_lo, il_t, LOFF[w])
                r0 = w * 128
                nc.vector.tensor_add(aggr_t[:, r0:r0 + 128], ps[:],
                                     xf_t[:, r0:r0 + 128])

            # Pass 2: hi rows + FFN + store.
            OGW = 8  # windows per output-store group
            out_g = None
            for w in range(NW):
                r0 = w * 128
                if w % OGW == 0:
                    gw = min(OGW, NW - w)
                    out_g = ogp.tile([H, OGW * 128], F16, tag="og")

                x16_t = ffnp.tile([128, 128], F16, tag="x16")
                if THI[w]:
                    ps = scatter_pass(w, THI[w], TLO[w], xpd_hi,
                                      ih_t, HOFF[w])
                    nc.vector.tensor_add(x16_t[:], ps[:],
                                         aggr_t[:, r0:r0 + 128])
                else:
                    nc.vector.tensor_copy(x16_t[:], aggr_t[:, r0:r0 + 128])

                ps2 = pffn.tile([128, 128], F32, tag="pffn")
                nc.tensor.matmul(ps2[:], w2t[:], x16_t[:],
                                 start=True, stop=True)
                y1_t = ffnp.tile([128, 128], F16, tag="y1")
                nc.scalar.activation(y1_t[:], ps2[:], act, bias=c2t[:, 0:1])
                ps3 = pffn.tile([128, 128], F32, tag="pffn")
                nc.tensor.matmul(ps3[:], w3t[:], y1_t[:],
                                 start=True, stop=True)
                y2_t = ffnp.tile([128, 128], F32, tag="y2")
                nc.scalar.activation(y2_t[:], ps3[:], act, bias=c3t[:, 0:1])
                oc = (w % OGW) * 128
                nc.vector.tensor_add(out_g[:, oc:oc + 128], y2_t[:],
                                     x16_t[:])
                if w % OGW == OGW - 1 or w == NW - 1:
                    g0 = (w // OGW) * OGW * 128
                    nc.sync.dma_start(outd[:, g0:g0 + gw * 128],
                                      out_g[:, :gw * 128])

    nc.compile()
    return nc


def run_compiled(nc, in_maps, meta, ncores=8, **kw):
    from concourse.bass_utils import run_bass_kernel_spmd
    res = run_bass_kernel_spmd(nc, in_maps, list(range(ncores)), **kw)
    N, NSH = meta["N"], meta["NSH"]
    out = np.empty((N, H), np.float32)
    for k in range(ncores):
        out[k * NSH:(k + 1) * NSH] = \
            res.results[k]["out"][:, :NSH].T.astype(np.float32)
    return out, res


def kernel(**inputs):
    inputs = {k: np.asarray(v) for k, v in inputs.items()}
    in_maps, meta = prep_inputs(**inputs)
    nc = build_program(meta)
    out, _ = run_compiled(nc, in_maps, meta)
    return out
